# revision 11
# baseline (speedup 1.0000x reference)
"""Trainium2 Bass kernel for nn_Cluster_7017976562037 (vq_codebook).

reference:
  labels = argmin_k ||x_s - c_k||^2          x: [B,S,D] f32, c: [K,D] f32
  mask[b,i,j] = (labels[b,i] == labels[b,j]) as f32
returns (mask [B,S,S] f32, labels [B,S] int32)

Data-parallel over batch: 8 cores x 4 batches. Host-side prep (layout
only): x is re-tiled into PE-ready transposed blocks, x_sq/c_sq are
computed with jax-CPU so the device d2 rounding bit-matches the
reference. Device per 128-sample tile: fp32 matmul G = x @ cT
(16 PSUM-accumulated chunk matmuls), ACT computes u = 2G - x_sq, DVE
adds -c_sq (u2 = -d2, exact fp32 mirror of the reference), then DVE
max-reduce + max_index give the argmin labels. The [S,S] mask is
tensor_scalar is_equal per 128-row tile on DVE against a label row
broadcast built by a PE ones-outer-product.
"""

import sys

if "/opt/trn_rl_repo" not in sys.path:
    sys.path.insert(0, "/opt/trn_rl_repo")

import numpy as np

B, S, D, K = 32, 2048, 2048, 256
N_CORES = 8
BPC = B // N_CORES          # batches per core
P = 128                     # partitions
ST = S // P                 # 16 s-tiles per batch
DC = D // P                 # 16 contraction chunks

_NC_CACHE = {}


def _build_nc(bpc=BPC, s=S, d=D, k=K):
    import concourse.bass as bass
    import concourse.tile as tile
    from concourse import bacc, mybir
    from concourse.masks import make_identity

    st, dc = s // P, d // P
    f32 = mybir.dt.float32
    i32 = mybir.dt.int32
    u32 = mybir.dt.uint32
    AF = mybir.ActivationFunctionType
    OP = mybir.AluOpType

    nc = bacc.Bacc("TRN2", target_bir_lowering=False, debug=False,
                   num_devices=N_CORES)

    # xT quad blocks: [bpc, nq, P, dc, 4, P] where
    # [q, p, c, f, m] = x[b, 128*(4q+f)+m, 128c+p]
    nq = st // 4
    xt_dram = nc.dram_tensor("xt", [bpc, nq, P, dc, 4, P], f32,
                             kind="ExternalInput")
    ct_dram = nc.dram_tensor("centersT", [d, k], f32, kind="ExternalInput")
    ncsq_dram = nc.dram_tensor("neg_c_sq", [P, k], f32, kind="ExternalInput")
    # neg_x_sq tiled: [bpc, P, st] where [p, t] = -x_sq[b, 128t+p]
    nxsq_dram = nc.dram_tensor("neg_x_sq", [bpc, P, st], f32,
                               kind="ExternalInput")
    mask_dram = nc.dram_tensor("state_weight", [bpc, s, s], f32,
                               kind="ExternalOutput")
    lab_dram = nc.dram_tensor("predicted_labels", [bpc, s], i32,
                              kind="ExternalOutput")

    with tile.TileContext(nc) as tc:
        with (
            tc.tile_pool(name="const", bufs=1) as const,
            tc.tile_pool(name="xt", bufs=2) as xtp,
            tc.tile_pool(name="gts", bufs=2) as gts,
            tc.tile_pool(name="small", bufs=6) as small,
            tc.tile_pool(name="lab", bufs=2) as labp,
            tc.tile_pool(name="mout", bufs=4) as mout,
            tc.tile_pool(name="px", bufs=1, space="PSUM") as px,
            tc.tile_pool(name="pg", bufs=4, space="PSUM") as pg,
            tc.tile_pool(name="pgt", bufs=1, space="PSUM") as pgt,
            tc.tile_pool(name="pl", bufs=1, space="PSUM") as pl,
        ):
            identity = const.tile([P, P], f32)
            make_identity(nc, identity[:])
            ct_sb = const.tile([P, dc, k], f32)
            nc.sync.dma_start(out=ct_sb[:],
                              in_=ct_dram.rearrange("(c p) k -> p c k", p=P))
            ncsq_sb = const.tile([P, k], f32)
            nc.sync.dma_start(out=ncsq_sb[:], in_=ncsq_dram[:])
            ones1 = const.tile([1, P], f32)
            nc.vector.memset(ones1[:], 1.0)

            for b in range(bpc):
                # ---------------- phase 1: labels ----------------
                nxsq = labp.tile([P, st], f32)
                nc.sync.dma_start(out=nxsq[:], in_=nxsq_dram[b])
                labels_colf = labp.tile([P, st], f32)
                for q in range(st // 4):
                    xq = xtp.tile([P, dc, 4 * P], f32)
                    nc.sync.dma_start(
                        out=xq[:],
                        in_=xt_dram[b, q].rearrange("p c f m -> p c (f m)"))
                    # GT = cT.T-chunks @ x-chunks: [2*128 k, 512 m] in PSUM
                    gt_ps = pgt.tile([P, 2, 512], f32)
                    for ci in range(dc):
                        for kh in range(2):
                            nc.tensor.matmul(
                                gt_ps[:, kh, :],
                                ct_sb[:, ci, P * kh:P * (kh + 1)],
                                xq[:, ci, :],
                                start=(ci == 0), stop=(ci == dc - 1))
                    gt_sb = gts.tile([P, 2, 512], f32)
                    nc.scalar.copy(gt_sb[:], gt_ps[:])
                    for f in range(4):
                        t = 4 * q + f
                        # transpose GT block back to [128 m, 256 k]
                        g_ps = pg.tile([P, k], f32)
                        for kh in range(2):
                            nc.tensor.transpose(
                                g_ps[:, P * kh:P * (kh + 1)],
                                gt_sb[:, kh, P * f:P * (f + 1)],
                                identity[:])
                        # u = fl(2G - x_sq)  (== -(fl(x_sq - 2G)) bit-exactly)
                        u = small.tile([P, k], f32)
                        nc.scalar.activation(u[:], g_ps[:], AF.Identity,
                                             bias=nxsq[:, t:t + 1], scale=2.0)
                        # u2 = fl(u - c_sq) = -d2 ; umax = max_k u2
                        u2 = small.tile([P, k], f32)
                        umax = small.tile([P, 1], f32)
                        nc.vector.tensor_tensor(u2[:], u[:], ncsq_sb[:],
                                                OP.add)
                        nc.vector.tensor_reduce(umax[:], u2[:],
                                                mybir.AxisListType.X, OP.max)
                        idx8 = small.tile([P, 8], u32)
                        um = umax[:]
                        um8 = bass.AP(um.tensor, um.offset,
                                      [um.ap[0], [0, 8]])
                        nc.vector.max_index(idx8[:], um8, u2[:])
                        nc.vector.tensor_copy(labels_colf[:, t:t + 1],
                                              idx8[:, 0:1])

                # ---- labels row, int32 out, broadcast to all partitions ----
                ps_lab = pl.tile([st, P], f32)
                nc.tensor.transpose(ps_lab[:], labels_colf[:], identity[:])
                labT_f = labp.tile([st, P], f32)
                nc.scalar.copy(labT_f[:], ps_lab[:])
                labT_i = labp.tile([st, P], i32)
                nc.vector.tensor_copy(labT_i[:], ps_lab[:])
                nc.sync.dma_start(
                    out=lab_dram[b].rearrange("(t p) -> t p", t=st),
                    in_=labT_i[:])
                labT_row = labp.tile([1, s], f32)
                nc.gpsimd.dma_start(out=labT_row[:], in_=labT_f[:])
                lab_bcast = labp.tile([P, s], f32)
                for jg in range(s // 512):
                    ps_b = px.tile([P, 512], f32, tag="ps_b")
                    nc.tensor.matmul(ps_b[:], ones1[:],
                                     labT_row[0:1, 512 * jg:512 * (jg + 1)],
                                     start=True, stop=True)
                    nc.scalar.copy(lab_bcast[:, 512 * jg:512 * (jg + 1)],
                                   ps_b[:])

                # ---------------- phase 2: mask ----------------
                for t in range(st):
                    m_tile = mout.tile([P, s], f32)
                    nc.vector.tensor_scalar(m_tile[:], lab_bcast[:],
                                            labels_colf[:, t:t + 1], None,
                                            OP.is_equal)
                    nc.sync.dma_start(
                        out=mask_dram[b, P * t:P * (t + 1), :],
                        in_=m_tile[:])
    nc.compile()
    return nc


def _get_nc():
    key = (BPC, S, D, K)
    if key not in _NC_CACHE:
        _NC_CACHE[key] = _build_nc()
    return _NC_CACHE[key]


def _sum_sq_rows(a):
    """Row-wise sum of squares, matching the reference's jnp.sum(x*x, -1)
    bit-for-bit (jax CPU); numpy fallback if jax-cpu is unavailable."""
    try:
        import jax

        with jax.default_device(jax.devices("cpu")[0]):
            import jax.numpy as jnp

            ja = jnp.asarray(a)
            return np.asarray(jnp.sum(ja * ja, axis=-1))
    except Exception:
        return (a.astype(np.float32) ** 2).sum(axis=-1, dtype=np.float32)


def _prep_centers(centers):
    centersT = np.ascontiguousarray(centers.T).astype(np.float32, copy=False)
    c_sq = _sum_sq_rows(centers)
    neg_c_sq = np.ascontiguousarray(
        np.repeat((-c_sq.astype(np.float32))[None, :], P, axis=0))
    return centersT, neg_c_sq


def _prep_x(x):
    """x [b, s, d] -> (xt quads [b, nq, P, dc, 4, P], neg_x_sq [b, P, st])."""
    b, s, d = x.shape
    st, dc = s // P, d // P
    xt = np.ascontiguousarray(
        x.reshape(b, st // 4, 4, P, dc, P).transpose(0, 1, 5, 4, 2, 3))
    x_sq = _sum_sq_rows(x)                       # [b, s]
    nxsq = np.ascontiguousarray(
        (-x_sq.astype(np.float32)).reshape(b, st, P).transpose(0, 2, 1))
    return xt, nxsq


def kernel(learning_state, centers):
    from concourse.bass_utils import run_bass_kernel_spmd

    learning_state = np.asarray(learning_state, dtype=np.float32)
    centers = np.asarray(centers, dtype=np.float32)
    nc = _get_nc()
    centersT, neg_c_sq = _prep_centers(centers)
    in_maps = []
    for i in range(N_CORES):
        xs = learning_state[i * BPC:(i + 1) * BPC]
        xt, nxsq = _prep_x(xs)
        in_maps.append({
            "xt": xt,
            "centersT": centersT,
            "neg_c_sq": neg_c_sq,
            "neg_x_sq": nxsq,
        })
    res = run_bass_kernel_spmd(nc, in_maps, list(range(N_CORES)))
    mask = np.concatenate([res.results[i]["state_weight"]
                           for i in range(N_CORES)], axis=0)
    labels = np.concatenate([res.results[i]["predicted_labels"]
                             for i in range(N_CORES)], axis=0)
    return mask, labels


# revision 12
# speedup vs baseline: 1.0278x; 1.0278x over previous
"""Trainium2 Bass kernel for nn_Cluster_7017976562037 (vq_codebook).

reference:
  labels = argmin_k ||x_s - c_k||^2          x: [B,S,D] f32, c: [K,D] f32
  mask[b,i,j] = (labels[b,i] == labels[b,j]) as f32
returns (mask [B,S,S] f32, labels [B,S] int32)

Data-parallel over batch: 8 cores x 4 batches. Host-side prep (layout
only): x is re-tiled into PE-ready transposed blocks, x_sq/c_sq are
computed with jax-CPU so the device d2 rounding bit-matches the
reference. Device per 128-sample tile: fp32 matmul G = x @ cT
(16 PSUM-accumulated chunk matmuls), ACT computes u = 2G - x_sq, DVE
adds -c_sq (u2 = -d2, exact fp32 mirror of the reference), then DVE
max-reduce + max_index give the argmin labels. The [S,S] mask is
tensor_scalar is_equal per 128-row tile on DVE against a label row
broadcast built by a PE ones-outer-product.
"""

import sys

if "/opt/trn_rl_repo" not in sys.path:
    sys.path.insert(0, "/opt/trn_rl_repo")

import numpy as np

B, S, D, K = 32, 2048, 2048, 256
N_CORES = 8
BPC = B // N_CORES          # batches per core
P = 128                     # partitions
ST = S // P                 # 16 s-tiles per batch
DC = D // P                 # 16 contraction chunks

_NC_CACHE = {}


def _build_nc(bpc=BPC, s=S, d=D, k=K):
    import concourse.bass as bass
    import concourse.tile as tile
    from concourse import bacc, mybir
    from concourse.masks import make_identity

    st, dc = s // P, d // P
    f32 = mybir.dt.float32
    i32 = mybir.dt.int32
    u32 = mybir.dt.uint32
    AF = mybir.ActivationFunctionType
    OP = mybir.AluOpType

    nc = bacc.Bacc("TRN2", target_bir_lowering=False, debug=False,
                   num_devices=N_CORES)

    # xT quad blocks: [bpc, nq, P, dc, 4, P] where
    # [q, p, c, f, m] = x[b, 128*(4q+f)+m, 128c+p]
    nq = st // 4
    xt_dram = nc.dram_tensor("xt", [bpc, nq, P, dc, 4, P], f32,
                             kind="ExternalInput")
    ct_dram = nc.dram_tensor("centersT", [d, k], f32, kind="ExternalInput")
    ncsq_dram = nc.dram_tensor("neg_c_sq", [P, k], f32, kind="ExternalInput")
    # neg_x_sq tiled: [bpc, P, st] where [p, t] = -x_sq[b, 128t+p]
    nxsq_dram = nc.dram_tensor("neg_x_sq", [bpc, P, st], f32,
                               kind="ExternalInput")
    mask_dram = nc.dram_tensor("state_weight", [bpc, s, s], f32,
                               kind="ExternalOutput")
    lab_dram = nc.dram_tensor("predicted_labels", [bpc, s], i32,
                              kind="ExternalOutput")

    with tile.TileContext(nc) as tc:
        with (
            tc.tile_pool(name="const", bufs=1) as const,
            tc.tile_pool(name="xt", bufs=2) as xtp,
            tc.tile_pool(name="gts", bufs=2) as gts,
            tc.tile_pool(name="small", bufs=6) as small,
            tc.tile_pool(name="lab", bufs=2) as labp,
            tc.tile_pool(name="mout", bufs=4) as mout,
            tc.tile_pool(name="px", bufs=1, space="PSUM") as px,
            tc.tile_pool(name="pg", bufs=2, space="PSUM") as pg,
            tc.tile_pool(name="pgt", bufs=2, space="PSUM") as pgt,
            tc.tile_pool(name="pl", bufs=1, space="PSUM") as pl,
        ):
            identity = const.tile([P, P], f32)
            make_identity(nc, identity[:])
            ct_sb = const.tile([P, dc, k], f32)
            nc.sync.dma_start(out=ct_sb[:],
                              in_=ct_dram.rearrange("(c p) k -> p c k", p=P))
            ncsq_sb = const.tile([P, k], f32)
            nc.sync.dma_start(out=ncsq_sb[:], in_=ncsq_dram[:])
            ones1 = const.tile([1, P], f32)
            nc.vector.memset(ones1[:], 1.0)

            for b in range(bpc):
                # ---------------- phase 1: labels ----------------
                nxsq = labp.tile([P, st], f32)
                nc.sync.dma_start(out=nxsq[:], in_=nxsq_dram[b])
                labels_colf = labp.tile([P, st], f32)
                for q in range(st // 4):
                    xq = xtp.tile([P, dc, 4 * P], f32)
                    nc.sync.dma_start(
                        out=xq[:],
                        in_=xt_dram[b, q].rearrange("p c f m -> p c (f m)"))
                    # GT = cT.T-chunks @ x-chunks: [2*128 k, 512 m] in PSUM
                    gt_ps = pgt.tile([P, 2, 512], f32)
                    for ci in range(dc):
                        for kh in range(2):
                            nc.tensor.matmul(
                                gt_ps[:, kh, :],
                                ct_sb[:, ci, P * kh:P * (kh + 1)],
                                xq[:, ci, :],
                                start=(ci == 0), stop=(ci == dc - 1))
                    gt_sb = gts.tile([P, 2, 512], f32)
                    nc.scalar.copy(gt_sb[:], gt_ps[:])
                    for f in range(4):
                        t = 4 * q + f
                        # transpose GT block back to [128 m, 256 k]
                        g_ps = pg.tile([P, k], f32)
                        for kh in range(2):
                            nc.tensor.transpose(
                                g_ps[:, P * kh:P * (kh + 1)],
                                gt_sb[:, kh, P * f:P * (f + 1)],
                                identity[:])
                        # u = fl(2G - x_sq)  (== -(fl(x_sq - 2G)) bit-exactly)
                        u = small.tile([P, k], f32)
                        nc.scalar.activation(u[:], g_ps[:], AF.Identity,
                                             bias=nxsq[:, t:t + 1], scale=2.0)
                        # u2 = fl(u - c_sq) = -d2 ; umax = max_k u2
                        u2 = small.tile([P, k], f32)
                        umax = small.tile([P, 1], f32)
                        nc.vector.tensor_tensor(u2[:], u[:], ncsq_sb[:],
                                                OP.add)
                        nc.vector.tensor_reduce(umax[:], u2[:],
                                                mybir.AxisListType.X, OP.max)
                        idx8 = small.tile([P, 8], u32)
                        um = umax[:]
                        um8 = bass.AP(um.tensor, um.offset,
                                      [um.ap[0], [0, 8]])
                        nc.vector.max_index(idx8[:], um8, u2[:])
                        nc.vector.tensor_copy(labels_colf[:, t:t + 1],
                                              idx8[:, 0:1])

                # ---- labels row, int32 out, broadcast to all partitions ----
                ps_lab = pl.tile([st, P], f32)
                nc.tensor.transpose(ps_lab[:], labels_colf[:], identity[:])
                labT_f = labp.tile([st, P], f32)
                nc.scalar.copy(labT_f[:], ps_lab[:])
                labT_i = labp.tile([st, P], i32)
                nc.vector.tensor_copy(labT_i[:], ps_lab[:])
                nc.sync.dma_start(
                    out=lab_dram[b].rearrange("(t p) -> t p", t=st),
                    in_=labT_i[:])
                labT_row = labp.tile([1, s], f32)
                nc.gpsimd.dma_start(out=labT_row[:], in_=labT_f[:])
                lab_bcast = labp.tile([P, s], f32)
                for jg in range(s // 512):
                    ps_b = px.tile([P, 512], f32, tag="ps_b")
                    nc.tensor.matmul(ps_b[:], ones1[:],
                                     labT_row[0:1, 512 * jg:512 * (jg + 1)],
                                     start=True, stop=True)
                    nc.scalar.copy(lab_bcast[:, 512 * jg:512 * (jg + 1)],
                                   ps_b[:])

                # ---------------- phase 2: mask ----------------
                for t in range(st):
                    m_tile = mout.tile([P, s], f32)
                    nc.vector.tensor_scalar(m_tile[:], lab_bcast[:],
                                            labels_colf[:, t:t + 1], None,
                                            OP.is_equal)
                    nc.sync.dma_start(
                        out=mask_dram[b, P * t:P * (t + 1), :],
                        in_=m_tile[:])
    nc.compile()
    return nc


def _get_nc():
    key = (BPC, S, D, K)
    if key not in _NC_CACHE:
        _NC_CACHE[key] = _build_nc()
    return _NC_CACHE[key]


def _sum_sq_rows(a):
    """Row-wise sum of squares, matching the reference's jnp.sum(x*x, -1)
    bit-for-bit (jax CPU); numpy fallback if jax-cpu is unavailable."""
    try:
        import jax

        with jax.default_device(jax.devices("cpu")[0]):
            import jax.numpy as jnp

            ja = jnp.asarray(a)
            return np.asarray(jnp.sum(ja * ja, axis=-1))
    except Exception:
        return (a.astype(np.float32) ** 2).sum(axis=-1, dtype=np.float32)


def _prep_centers(centers):
    centersT = np.ascontiguousarray(centers.T).astype(np.float32, copy=False)
    c_sq = _sum_sq_rows(centers)
    neg_c_sq = np.ascontiguousarray(
        np.repeat((-c_sq.astype(np.float32))[None, :], P, axis=0))
    return centersT, neg_c_sq


def _prep_x(x):
    """x [b, s, d] -> (xt quads [b, nq, P, dc, 4, P], neg_x_sq [b, P, st])."""
    b, s, d = x.shape
    st, dc = s // P, d // P
    xt = np.ascontiguousarray(
        x.reshape(b, st // 4, 4, P, dc, P).transpose(0, 1, 5, 4, 2, 3))
    x_sq = _sum_sq_rows(x)                       # [b, s]
    nxsq = np.ascontiguousarray(
        (-x_sq.astype(np.float32)).reshape(b, st, P).transpose(0, 2, 1))
    return xt, nxsq


def kernel(learning_state, centers):
    from concourse.bass_utils import run_bass_kernel_spmd

    learning_state = np.asarray(learning_state, dtype=np.float32)
    centers = np.asarray(centers, dtype=np.float32)
    nc = _get_nc()
    centersT, neg_c_sq = _prep_centers(centers)
    in_maps = []
    for i in range(N_CORES):
        xs = learning_state[i * BPC:(i + 1) * BPC]
        xt, nxsq = _prep_x(xs)
        in_maps.append({
            "xt": xt,
            "centersT": centersT,
            "neg_c_sq": neg_c_sq,
            "neg_x_sq": nxsq,
        })
    res = run_bass_kernel_spmd(nc, in_maps, list(range(N_CORES)))
    mask = np.concatenate([res.results[i]["state_weight"]
                           for i in range(N_CORES)], axis=0)
    labels = np.concatenate([res.results[i]["predicted_labels"]
                             for i in range(N_CORES)], axis=0)
    return mask, labels


# revision 13
# speedup vs baseline: 1.1122x; 1.0821x over previous
"""Trainium2 Bass kernel for nn_Cluster_7017976562037 (vq_codebook).

reference:
  labels = argmin_k ||x_s - c_k||^2          x: [B,S,D] f32, c: [K,D] f32
  mask[b,i,j] = (labels[b,i] == labels[b,j]) as f32
returns (mask [B,S,S] f32, labels [B,S] int32)

Data-parallel over batch: 8 cores x 4 batches. Host-side prep (layout
only): x is re-tiled into PE-ready transposed blocks, x_sq/c_sq are
computed with jax-CPU so the device d2 rounding bit-matches the
reference. Device per 128-sample tile: fp32 matmul G = x @ cT
(16 PSUM-accumulated chunk matmuls), ACT computes u = 2G - x_sq, DVE
adds -c_sq (u2 = -d2, exact fp32 mirror of the reference), then DVE
max-reduce + max_index give the argmin labels. The [S,S] mask is
tensor_scalar is_equal per 128-row tile on DVE against a label row
broadcast built by a PE ones-outer-product.
"""

import sys

if "/opt/trn_rl_repo" not in sys.path:
    sys.path.insert(0, "/opt/trn_rl_repo")

import numpy as np

B, S, D, K = 32, 2048, 2048, 256
N_CORES = 8
BPC = B // N_CORES          # batches per core
P = 128                     # partitions
ST = S // P                 # 16 s-tiles per batch
DC = D // P                 # 16 contraction chunks

_NC_CACHE = {}


def _build_nc(bpc=BPC, s=S, d=D, k=K):
    import concourse.bass as bass
    import concourse.tile as tile
    from concourse import bacc, mybir
    from concourse.masks import make_identity

    st, dc = s // P, d // P
    f32 = mybir.dt.float32
    i32 = mybir.dt.int32
    u32 = mybir.dt.uint32
    AF = mybir.ActivationFunctionType
    OP = mybir.AluOpType

    nc = bacc.Bacc("TRN2", target_bir_lowering=False, debug=False,
                   num_devices=N_CORES)

    # xT quad blocks: [bpc, nq, P, dc, 4, P] where
    # [q, p, c, f, m] = x[b, 128*(4q+f)+m, 128c+p]
    nq = st // 4
    xt_dram = nc.dram_tensor("xt", [bpc, nq, P, dc, 4, P], f32,
                             kind="ExternalInput")
    ct_dram = nc.dram_tensor("centersT", [d, k], f32, kind="ExternalInput")
    ncsq_dram = nc.dram_tensor("neg_c_sq", [P, k], f32, kind="ExternalInput")
    # neg_x_sq tiled: [bpc, P, st] where [p, t] = -x_sq[b, 128t+p]
    nxsq_dram = nc.dram_tensor("neg_x_sq", [bpc, P, st], f32,
                               kind="ExternalInput")
    mask_dram = nc.dram_tensor("state_weight", [bpc, s, s], f32,
                               kind="ExternalOutput")
    lab_dram = nc.dram_tensor("predicted_labels", [bpc, s], i32,
                              kind="ExternalOutput")

    with tile.TileContext(nc) as tc:
        with (
            tc.tile_pool(name="const", bufs=1) as const,
            tc.tile_pool(name="xt", bufs=3) as xtp,
            tc.tile_pool(name="gts", bufs=2) as gts,
            tc.tile_pool(name="small", bufs=6) as small,
            tc.tile_pool(name="lab", bufs=2) as labp,
            tc.tile_pool(name="mout", bufs=4) as mout,
            tc.tile_pool(name="px", bufs=1, space="PSUM") as px,
            tc.tile_pool(name="pg", bufs=2, space="PSUM") as pg,
            tc.tile_pool(name="pgt", bufs=2, space="PSUM") as pgt,
            tc.tile_pool(name="pl", bufs=1, space="PSUM") as pl,
        ):
            identity = const.tile([P, P], f32)
            make_identity(nc, identity[:])
            ct_sb = const.tile([P, dc, k], f32)
            nc.sync.dma_start(out=ct_sb[:],
                              in_=ct_dram.rearrange("(c p) k -> p c k", p=P))
            ncsq_sb = const.tile([P, k], f32)
            nc.sync.dma_start(out=ncsq_sb[:], in_=ncsq_dram[:])
            ones1 = const.tile([1, P], f32)
            nc.vector.memset(ones1[:], 1.0)

            for b in range(bpc):
                # ---------------- phase 1: labels ----------------
                nxsq = labp.tile([P, st], f32)
                nc.sync.dma_start(out=nxsq[:], in_=nxsq_dram[b])
                labels_colf = labp.tile([P, st], f32)
                for q in range(st // 4):
                    xq = xtp.tile([P, dc, 4 * P], f32)
                    nc.sync.dma_start(
                        out=xq[:],
                        in_=xt_dram[b, q].rearrange("p c f m -> p c (f m)"))
                    # GT = cT.T-chunks @ x-chunks: [2*128 k, 512 m] in PSUM
                    gt_ps = pgt.tile([P, 2, 512], f32)
                    for ci in range(dc):
                        for kh in range(2):
                            nc.tensor.matmul(
                                gt_ps[:, kh, :],
                                ct_sb[:, ci, P * kh:P * (kh + 1)],
                                xq[:, ci, :],
                                start=(ci == 0), stop=(ci == dc - 1))
                    gt_sb = gts.tile([P, 2, 512], f32)
                    nc.scalar.copy(gt_sb[:], gt_ps[:])
                    for f in range(4):
                        t = 4 * q + f
                        # transpose GT block back to [128 m, 256 k]
                        g_ps = pg.tile([P, k], f32)
                        for kh in range(2):
                            nc.tensor.transpose(
                                g_ps[:, P * kh:P * (kh + 1)],
                                gt_sb[:, kh, P * f:P * (f + 1)],
                                identity[:])
                        # u = fl(2G - x_sq)  (== -(fl(x_sq - 2G)) bit-exactly)
                        u = small.tile([P, k], f32)
                        nc.scalar.activation(u[:], g_ps[:], AF.Identity,
                                             bias=nxsq[:, t:t + 1], scale=2.0)
                        # u2 = fl(u - c_sq) = -d2 ; umax = max_k u2
                        u2 = small.tile([P, k], f32)
                        umax = small.tile([P, 1], f32)
                        nc.vector.tensor_tensor(u2[:], u[:], ncsq_sb[:],
                                                OP.add)
                        nc.vector.tensor_reduce(umax[:], u2[:],
                                                mybir.AxisListType.X, OP.max)
                        idx8 = small.tile([P, 8], u32)
                        um = umax[:]
                        um8 = bass.AP(um.tensor, um.offset,
                                      [um.ap[0], [0, 8]])
                        nc.vector.max_index(idx8[:], um8, u2[:])
                        nc.vector.tensor_copy(labels_colf[:, t:t + 1],
                                              idx8[:, 0:1])

                # ---- labels row, int32 out, broadcast to all partitions ----
                ps_lab = pl.tile([st, P], f32)
                nc.tensor.transpose(ps_lab[:], labels_colf[:], identity[:])
                labT_f = labp.tile([st, P], f32)
                nc.scalar.copy(labT_f[:], ps_lab[:])
                labT_i = labp.tile([st, P], i32)
                nc.vector.tensor_copy(labT_i[:], ps_lab[:])
                nc.sync.dma_start(
                    out=lab_dram[b].rearrange("(t p) -> t p", t=st),
                    in_=labT_i[:])
                labT_row = labp.tile([1, s], f32)
                nc.gpsimd.dma_start(out=labT_row[:], in_=labT_f[:])
                lab_bcast = labp.tile([P, s], f32)
                for jg in range(s // 512):
                    ps_b = px.tile([P, 512], f32, tag="ps_b")
                    nc.tensor.matmul(ps_b[:], ones1[:],
                                     labT_row[0:1, 512 * jg:512 * (jg + 1)],
                                     start=True, stop=True)
                    nc.scalar.copy(lab_bcast[:, 512 * jg:512 * (jg + 1)],
                                   ps_b[:])

                # ---------------- phase 2: mask ----------------
                for t in range(st):
                    m_tile = mout.tile([P, s], f32)
                    nc.vector.tensor_scalar(m_tile[:], lab_bcast[:],
                                            labels_colf[:, t:t + 1], None,
                                            OP.is_equal)
                    nc.sync.dma_start(
                        out=mask_dram[b, P * t:P * (t + 1), :],
                        in_=m_tile[:])
    nc.compile()
    return nc


def _get_nc():
    key = (BPC, S, D, K)
    if key not in _NC_CACHE:
        _NC_CACHE[key] = _build_nc()
    return _NC_CACHE[key]


def _sum_sq_rows(a):
    """Row-wise sum of squares, matching the reference's jnp.sum(x*x, -1)
    bit-for-bit (jax CPU); numpy fallback if jax-cpu is unavailable."""
    try:
        import jax

        with jax.default_device(jax.devices("cpu")[0]):
            import jax.numpy as jnp

            ja = jnp.asarray(a)
            return np.asarray(jnp.sum(ja * ja, axis=-1))
    except Exception:
        return (a.astype(np.float32) ** 2).sum(axis=-1, dtype=np.float32)


def _prep_centers(centers):
    centersT = np.ascontiguousarray(centers.T).astype(np.float32, copy=False)
    c_sq = _sum_sq_rows(centers)
    neg_c_sq = np.ascontiguousarray(
        np.repeat((-c_sq.astype(np.float32))[None, :], P, axis=0))
    return centersT, neg_c_sq


def _prep_x(x):
    """x [b, s, d] -> (xt quads [b, nq, P, dc, 4, P], neg_x_sq [b, P, st])."""
    b, s, d = x.shape
    st, dc = s // P, d // P
    xt = np.ascontiguousarray(
        x.reshape(b, st // 4, 4, P, dc, P).transpose(0, 1, 5, 4, 2, 3))
    x_sq = _sum_sq_rows(x)                       # [b, s]
    nxsq = np.ascontiguousarray(
        (-x_sq.astype(np.float32)).reshape(b, st, P).transpose(0, 2, 1))
    return xt, nxsq


def kernel(learning_state, centers):
    from concourse.bass_utils import run_bass_kernel_spmd

    learning_state = np.asarray(learning_state, dtype=np.float32)
    centers = np.asarray(centers, dtype=np.float32)
    nc = _get_nc()
    centersT, neg_c_sq = _prep_centers(centers)
    in_maps = []
    for i in range(N_CORES):
        xs = learning_state[i * BPC:(i + 1) * BPC]
        xt, nxsq = _prep_x(xs)
        in_maps.append({
            "xt": xt,
            "centersT": centersT,
            "neg_c_sq": neg_c_sq,
            "neg_x_sq": nxsq,
        })
    res = run_bass_kernel_spmd(nc, in_maps, list(range(N_CORES)))
    mask = np.concatenate([res.results[i]["state_weight"]
                           for i in range(N_CORES)], axis=0)
    labels = np.concatenate([res.results[i]["predicted_labels"]
                             for i in range(N_CORES)], axis=0)
    return mask, labels
